# revision 1
# baseline (speedup 1.0000x reference)
"""
Trainium2 Bass kernel for nn_CharNNClassifier:
    out = LSTM(emb[x])[:, -1] @ W_out.T + b_out      (B=256, S=512, V=256, E=128, H=256, O=128)

Strategy (v2)
-------------
Data-parallel over batch: 8 cores x 32 rows, no collectives.

Host-side weight prep (x-independent):
  * input projection collapses to a row gather of
    proj = emb @ W_ih.T + (b_ih + b_hh)   [V=256, 4H=1024]
  * gate order [g, f, i, o]; g rows scaled x2 so tanh(g) = 2*sigmoid(2g)-1
    folds into one big sigmoid over g,f,i (and o off-path).

Device-side per step, all laid out gates-on-partitions [128p x cols]:
  * pre is added into PSUM via 8 identity-stationary matmuls BEFORE h is
    ready (off the critical path); W_hh matmuls accumulate on top.
  * critical chain: 12 W-MMs (g,f,i tiles) -> ACT sigmoid [192 cols, psum src]
    -> DVE stt: uv = ([K|s_g] - 1/2) * [s_f|s_i] -> DVE stt: K' = (v + 1/2) + u
    -> ACT tanh(2K'-1) -> DVE h = s_o * t -> next step's MMs.
    Cell state kept as K = c/2 + 1/2; tanh(c) = Tanh(2K-1) via ACT input affine.
  * o-tile MMs + sigmoid(o) run behind the c-path.

Raw Bass (no Tile) with hand-placed semaphores; psum ping-pong across steps.
"""

import sys
import numpy as np
import ml_dtypes

for _p in ("/opt/trn_rl_repo", "/opt/trn_rl_repo/concourse"):
    if _p not in sys.path:
        sys.path.append(_p)

B, S, V, E, H, O = 256, 512, 256, 128, 256, 128
NCORES = 8
BC = B // NCORES          # 32 batch rows per core
CH = 4                    # steps per gather chunk (num_idxs = CH*BC = 128)
PRE_BUFS = 4              # gather ring depth

# gate permutation: new row order [g, f, i, o] (old order is i, f, g, o)
PERM = np.concatenate(
    [np.arange(512, 768), np.arange(256, 512), np.arange(0, 256), np.arange(768, 1024)]
)

BF16 = ml_dtypes.bfloat16


def build_nc(n_steps=S):
    import concourse.bacc as bacc
    import concourse.mybir as mybir
    from contextlib import ExitStack

    f32 = mybir.dt.float32
    bf16 = mybir.dt.bfloat16
    AF = mybir.ActivationFunctionType
    OP = mybir.AluOpType

    nch = n_steps // CH
    nc = bacc.Bacc("TRN2")

    whT_d = nc.dram_tensor("whT", [H, 1024], bf16, kind="ExternalInput")
    ident_d = nc.dram_tensor("ident", [128, 128], bf16, kind="ExternalInput")
    wout_d = nc.dram_tensor("woutT", [H, O], f32, kind="ExternalInput")
    bout_d = nc.dram_tensor("bout", [O, 1], f32, kind="ExternalInput")
    # pre-gathered input projections, one [128, 8, CH*BC] tile per chunk of
    # CH steps (host does the vocab gather; kernel just streams it in)
    preall_d = nc.dram_tensor(
        "preall", [nch, 128, 8, CH * BC], bf16, kind="ExternalInput"
    )
    out_d = nc.dram_tensor("outT", [O, BC], f32, kind="ExternalOutput")

    ctx = ExitStack()
    with ctx:
        wh_sb = ctx.enter_context(nc.sbuf_tensor("wh_sb", [128, 2, 1024], bf16))
        id_sb = ctx.enter_context(nc.sbuf_tensor("id_sb", [128, 128], bf16))
        wo_sb = ctx.enter_context(nc.sbuf_tensor("wo_sb", [128, 2, O], f32))
        bo_sb = ctx.enter_context(nc.sbuf_tensor("bo_sb", [128, 1], f32))
        pre_sb = ctx.enter_context(
            nc.sbuf_tensor("pre_sb", [128, PRE_BUFS, 8, CH * BC], bf16)
        )
        # big: [K | s_g | s_f | s_i | s_o] slots of 2*BC=64 cols each, fp32
        big_sb = ctx.enter_context(nc.sbuf_tensor("big_sb", [128, 5, 2 * BC], f32))
        uv_sb = ctx.enter_context(nc.sbuf_tensor("uv_sb", [128, 4 * BC], f32))
        t_sb = ctx.enter_context(nc.sbuf_tensor("t_sb", [128, 2 * BC], bf16))
        so_sb = ctx.enter_context(nc.sbuf_tensor("so_sb", [128, 2 * BC], bf16))
        hT_sb = ctx.enter_context(nc.sbuf_tensor("hT_sb", [128, 2 * BC], bf16))
        h32_sb = ctx.enter_context(nc.sbuf_tensor("h32_sb", [128, 2 * BC], f32))
        outT_sb = ctx.enter_context(nc.sbuf_tensor("outT_sb", [O, BC], f32))
        neg1_sb = ctx.enter_context(nc.sbuf_tensor("neg1_sb", [128, 1], f32))
        ps0 = ctx.enter_context(nc.psum_tensor("ps0", [128, 512], f32))
        ps1 = ctx.enter_context(nc.psum_tensor("ps1", [128, 512], f32))
        pso0 = ctx.enter_context(nc.psum_tensor("pso0", [128, 512], f32))
        pso1 = ctx.enter_context(nc.psum_tensor("pso1", [128, 512], f32))
        psf = ctx.enter_context(nc.psum_tensor("psf", [128, 512], f32))

        s_dma = ctx.enter_context(nc.semaphore("s_dma"))
        s_ld_id = ctx.enter_context(nc.semaphore("s_ld_id"))
        s_ld_wh = ctx.enter_context(nc.semaphore("s_ld_wh"))
        s_ld_out = ctx.enter_context(nc.semaphore("s_ld_out"))
        s_gat = [
            ctx.enter_context(nc.semaphore(f"s_gat{i}")) for i in range(PRE_BUFS)
        ]
        s_mm_gfi = ctx.enter_context(nc.semaphore("s_mm_gfi"))
        s_mm_o = ctx.enter_context(nc.semaphore("s_mm_o"))
        s_sig = ctx.enter_context(nc.semaphore("s_sig"))
        s_pfree = ctx.enter_context(nc.semaphore("s_pfree"))
        s_K = ctx.enter_context(nc.semaphore("s_K"))
        s_t = ctx.enter_context(nc.semaphore("s_t"))
        s_h0 = ctx.enter_context(nc.semaphore("s_h0"))
        s_h1 = ctx.enter_context(nc.semaphore("s_h1"))
        s_fin = ctx.enter_context(nc.semaphore("s_fin"))

        # ---------------- single block: DMAs + init + main loop ----------------
        # No all-engine barrier: each engine waits only on the semaphores it
        # actually needs, so gathers/compute start as soon as their inputs land.
        with nc.Block() as blk:

            @blk.sync
            def _(sync):
                # order: first pre-MM inputs land first so step 0 starts early
                sync.dma_start(id_sb[:], ident_d[:]).then_inc(s_dma, 16)
                sync.dma_start(pre_sb[:, 0], preall_d[0]).then_inc(s_gat[0], 16)
                sync.dma_start(
                    wh_sb[:], whT_d[:].rearrange("(k p) f -> p k f", p=128)
                ).then_inc(s_dma, 16)
                sync.dma_start(bo_sb[:], bout_d[:]).then_inc(s_dma, 16)
                sync.dma_start(
                    wo_sb[:], wout_d[:].rearrange("(k p) f -> p k f", p=128)
                ).then_inc(s_dma, 16)
                # stream remaining pre chunks through the 4-buffer ring
                for q in range(1, nch):
                    if q >= PRE_BUFS:
                        # pre buf q-PRE_BUFS fully consumed once the pre-MMs
                        # of its last step have run; those precede that
                        # step's o-MMs in the tensor queue, so s_mm_o of that
                        # step is a safe proxy.
                        sync.wait_ge(s_mm_o, CH * (q - PRE_BUFS) + CH)
                    sync.dma_start(pre_sb[:, q % PRE_BUFS], preall_d[q]).then_inc(
                        s_gat[q % PRE_BUFS], 16
                    )
                sync.wait_ge(s_fin, 2)
                sync.dma_start(out_d[:], outT_sb[:]).then_inc(s_dma, 16)
                sync.wait_ge(s_dma, 80)

            @blk.tensor
            def _(t):
                t.wait_ge(s_dma, 16)  # id_sb landed (pre-MMs only need this)
                for s in range(n_steps):
                    ps = ps0 if s % 2 == 0 else ps1
                    pso = pso0 if s % 2 == 0 else pso1
                    q = s // CH
                    # pre-add MMs: identity stationary, rhs = pre tile.
                    # Off the critical path (do not depend on h).
                    if s >= 2:
                        t.wait_ge(s_pfree, s - 1)
                    t.wait_ge(s_gat[q % PRE_BUFS], 16 * (q // PRE_BUFS + 1))
                    for m in range(8):
                        dst = (
                            ps[:, 32 * m : 32 * m + 32]
                            if m < 6
                            else pso[:, 32 * (m - 6) : 32 * (m - 6) + 32]
                        )
                        # start=True clears has_written for the WHOLE bank, so
                        # only the first pre-MM per bank may set it; the rest
                        # overwrite via cleared bits (start=False).
                        t.matmul(
                            dst,
                            id_sb[:],
                            pre_sb[
                                :, q % PRE_BUFS, m, BC * (s % CH) : BC * (s % CH) + BC
                            ],
                            start=(m == 0 or m == 6),
                            stop=False,
                        )
                    # W-MMs, k-major: k0 tiles as soon as the low half of h
                    # is ready, k1 after the high half.  g,f,i (m=0..5) then
                    # o tiles (m=6,7).
                    if s == 0:
                        t.wait_ge(s_dma, 32)  # wh_sb landed
                    t.wait_ge(s_h0, s + 1)
                    for m in range(6):
                        t.matmul(
                            ps[:, 32 * m : 32 * m + 32],
                            wh_sb[:, 0, 128 * m : 128 * m + 128],
                            hT_sb[:, 0:32],
                            start=False,
                            stop=False,
                        )
                    t.wait_ge(s_h1, s + 1)
                    last = None
                    for m in range(6):
                        last = t.matmul(
                            ps[:, 32 * m : 32 * m + 32],
                            wh_sb[:, 1, 128 * m : 128 * m + 128],
                            hT_sb[:, 32:64],
                            start=False,
                            stop=True,
                        )
                    last.then_inc(s_mm_gfi, 1)
                    for m in range(6, 8):
                        for k in range(2):
                            last = t.matmul(
                                pso[:, 32 * (m - 6) : 32 * (m - 6) + 32],
                                wh_sb[:, k, 128 * m : 128 * m + 128],
                                hT_sb[:, 32 * k : 32 * k + 32],
                                start=False,
                                stop=(k == 1),
                            )
                    last.then_inc(s_mm_o, 1)
                # final projection out^T = W_out @ h  (fp32)
                t.wait_ge(s_dma, 64)  # wo_sb / bo_sb landed
                t.wait_ge(s_h1, n_steps + 1)
                for k in range(2):
                    last = t.matmul(
                        psf[:O, 0:BC],
                        wo_sb[:, k, :],
                        h32_sb[:, 32 * k : 32 * k + 32],
                        start=(k == 0),
                        stop=(k == 1),
                    )
                last.then_inc(s_fin, 1)

            @blk.scalar
            def _(sc):
                for s in range(n_steps):
                    ps = ps0 if s % 2 == 0 else ps1
                    pso = pso0 if s % 2 == 0 else pso1
                    sc.wait_ge(s_mm_gfi, s + 1)
                    # sigmoid over g,f,i tiles -> big slots 1..3
                    sc.activation(
                        big_sb[:, 1:4].rearrange("p a b -> p (a b)"),
                        ps[:, 0 : 6 * BC],
                        AF.Sigmoid,
                    ).then_inc(s_sig, 1)
                    sc.wait_ge(s_mm_o, s + 1)
                    sc.activation(
                        so_sb[:], pso[:, 0 : 2 * BC], AF.Sigmoid
                    ).then_inc(s_pfree, 1)
                    sc.wait_ge(s_K, s + 1)
                    # tanh(c) = Tanh(2K - 1)
                    sc.activation(
                        t_sb[:], big_sb[:, 0], AF.Tanh, bias=neg1_sb[:, 0:1], scale=2.0
                    ).then_inc(s_t, 1)
                # final bias add
                sc.wait_ge(s_fin, 1)
                sc.activation(
                    outT_sb[:], psf[:O, 0:BC], AF.Identity, bias=bo_sb[:O, 0:1]
                ).then_inc(s_fin, 1)

            @blk.vector
            def _(v):
                v.memset(big_sb[:, 0], 0.5)  # K = c/2 + 1/2, c0 = 0
                v.memset(neg1_sb[:], -1.0)
                v.memset(hT_sb[:, 0:32], 0).then_inc(s_h0, 1)
                v.memset(hT_sb[:, 32:64], 0).then_inc(s_h1, 1)
                for s in range(n_steps):
                    v.wait_ge(s_sig, s + 1)
                    # uv = ([K | s_g] - 1/2) * [s_f | s_i]
                    v.scalar_tensor_tensor(
                        uv_sb[:],
                        big_sb[:, 0:2].rearrange("p a b -> p (a b)"),
                        0.5,
                        big_sb[:, 2:4].rearrange("p a b -> p (a b)"),
                        OP.subtract,
                        OP.mult,
                    )
                    # K' = (v + 1/2) + u
                    v.scalar_tensor_tensor(
                        big_sb[:, 0],
                        uv_sb[:, 0 : 2 * BC],
                        0.5,
                        uv_sb[:, 2 * BC : 4 * BC],
                        OP.add,
                        OP.add,
                    ).then_inc(s_K, 1)
                    v.wait_ge(s_t, s + 1)
                    if s < n_steps - 1:
                        v.tensor_tensor(
                            hT_sb[:, 0:32], so_sb[:, 0:32], t_sb[:, 0:32], OP.mult
                        ).then_inc(s_h0, 1)
                        v.tensor_tensor(
                            hT_sb[:, 32:64], so_sb[:, 32:64], t_sb[:, 32:64], OP.mult
                        ).then_inc(s_h1, 1)
                    else:
                        v.tensor_tensor(
                            h32_sb[:], so_sb[:], t_sb[:], OP.mult
                        ).then_inc(s_h1, 1)

        nc.compile()
    return nc


def make_preall(x_core, proj_bf, n_steps=S):
    """x_core [BC, n_steps] -> preall [nch, 128, 8, CH*BC] bf16.

    preall[q, p, m, sl*BC+b] = proj_bf[x_core[b, CH*q+sl], 128*m+p]
    i.e. the pre tile each pre-MM consumes, already in rhs layout."""
    nch = n_steps // CH
    arr = proj_bf[x_core]                       # [BC, S, 1024]
    arr = arr.reshape(BC, nch, CH, 8, 128)      # [b, q, sl, m, p]
    arr = arr.transpose(1, 4, 3, 2, 0)          # [q, p, m, sl, b]
    return np.ascontiguousarray(arr.reshape(nch, 128, 8, CH * BC))


def host_prep(x, emb, W_ih, W_hh, b_ih, b_hh, W_out, b_out, n_steps=S):
    x = np.asarray(x).astype(np.int64)
    emb = np.asarray(emb, dtype=np.float32)
    W_ih = np.asarray(W_ih, dtype=np.float32)
    W_hh = np.asarray(W_hh, dtype=np.float32)
    b_ih = np.asarray(b_ih, dtype=np.float32)
    b_hh = np.asarray(b_hh, dtype=np.float32)
    W_out = np.asarray(W_out, dtype=np.float32)
    b_out = np.asarray(b_out, dtype=np.float32)

    proj = emb @ W_ih.T + (b_ih + b_hh)          # [V, 4H]
    projP = proj[:, PERM]
    whP = W_hh[PERM]
    # g rows (first 256 in PERM order) scaled x2: tanh(g) = 2*sigmoid(2g)-1
    scale = np.ones((1024, 1), np.float32)
    scale[0:256] = 2.0
    projP = projP * scale.T
    whP = whP * scale

    proj_bf = np.ascontiguousarray(projP).astype(BF16)
    whT_bf = np.ascontiguousarray(whP.T).astype(BF16)   # [H, 4H]
    ident = np.eye(128, dtype=np.float32).astype(BF16)
    woutT = np.ascontiguousarray(W_out.T).astype(np.float32)   # [H, O]
    bout = np.ascontiguousarray(b_out.reshape(O, 1)).astype(np.float32)

    in_maps = []
    for c in range(NCORES):
        x_core = x[BC * c : BC * (c + 1), :n_steps]
        in_maps.append(
            {
                "whT": whT_bf,
                "ident": ident,
                "woutT": woutT,
                "bout": bout,
                "preall": make_preall(x_core, proj_bf, n_steps),
            }
        )
    return in_maps


_NC_CACHE = {}


def _get_nc(n_steps=S):
    if n_steps not in _NC_CACHE:
        _NC_CACHE[n_steps] = build_nc(n_steps)
    return _NC_CACHE[n_steps]


def kernel(x, emb, W_ih, W_hh, b_ih, b_hh, W_out, b_out, trace=False):
    from concourse.bass_utils import run_bass_kernel_spmd

    in_maps = host_prep(x, emb, W_ih, W_hh, b_ih, b_hh, W_out, b_out)
    nc = _get_nc(S)
    res = run_bass_kernel_spmd(nc, in_maps, core_ids=list(range(NCORES)), trace=trace)
    out = np.empty((B, O), dtype=np.float32)
    for c in range(NCORES):
        out[BC * c : BC * (c + 1), :] = res.results[c]["outT"].T
    kernel.last_results = res
    return out

